# revision 1
# baseline (speedup 1.0000x reference)
"""Self-contained Trainium2 Bass kernel for nn_MultiHeadAttention_68367289417808.

kernel(**inputs) takes FULL unsharded inputs (as in reference.setup_inputs())
and returns the FULL [4, 2048, 1024] output.

Sharding: 8 cores = (batch 4) x (query-half 2); no collectives needed.
Each core runs the full per-shard MHA in fp32r (e8m11) on the tensor engine:
  - host pre-transposes/pre-rounds activations and weights
  - projections V->K->Q (DMA-balanced order) with fused bias
  - attention in transposed-scores layout [sk, sq]; masking done entirely on
    the tensor engine: scores += -1e4 * (1-mask) via a scaled-identity matmul
    (masked exp underflows to 0), then the masked weights are restored to
    ~exp(-1e-6)=1 by accumulating [vE|vO]^T @ (1-mask) value corrections
    (one M=128 matmul per head pair) plus a shared column-sum for the
    denominators; the softmax denominator rides along as a ones column.
  - output projection + bias; per-core [1024, 1024] slices assembled on host.
"""
import time

import jax
import numpy as np
from jax.experimental.shard_map import shard_map
from jax.sharding import Mesh, PartitionSpec

import concourse.bass as bass
import concourse.bacc as bacc
import concourse.mybir as mybir
import concourse.tile as tile
from concourse import bass2jax
from concourse.bass import ts, ds

F32 = mybir.dt.float32
F32R = mybir.dt.float32r
U8 = mybir.dt.uint8
AF = mybir.ActivationFunctionType
MULT = mybir.AluOpType.mult
ADD = mybir.AluOpType.add

P = 128
SQ = 1024
SK = 2048
D = 1024
H = 16
DK = 64
HK = 1024
BIG = 1.0e4


def round_f32r(a: np.ndarray) -> np.ndarray:
    """Round fp32 to fp32r (e8m11: keep 11 mantissa bits, RNE)."""
    a = np.ascontiguousarray(a, dtype=np.float32)
    u = a.view(np.uint32)
    add = np.uint32(0x7FF) + ((u >> np.uint32(12)) & np.uint32(1))
    u2 = (u + add) & np.uint32(0xFFFFF000)
    return u2.view(np.float32)


def build_mha():
    nc = bacc.Bacc("TRN2", target_bir_lowering=False)

    qT = nc.dram_tensor("qT", [D, SQ], F32R, kind="ExternalInput")
    kT = nc.dram_tensor("kT", [D, SK], F32R, kind="ExternalInput")
    vT = nc.dram_tensor("vT", [D, SK], F32R, kind="ExternalInput")
    mcT = nc.dram_tensor("mcT", [SK, SQ], U8, kind="ExternalInput")
    wq = nc.dram_tensor("wq", [D, HK], F32R, kind="ExternalInput")
    wk = nc.dram_tensor("wk", [D, HK], F32R, kind="ExternalInput")
    wv = nc.dram_tensor("wv", [D, HK], F32R, kind="ExternalInput")
    wo = nc.dram_tensor("wo", [HK, D], F32R, kind="ExternalInput")
    bq2 = nc.dram_tensor("bq2", [P, 8], F32, kind="ExternalInput")
    bk2 = nc.dram_tensor("bk2", [P, 8], F32, kind="ExternalInput")
    bvr = nc.dram_tensor("bvr", [1, HK], F32R, kind="ExternalInput")
    bor = nc.dram_tensor("bor", [1, D], F32R, kind="ExternalInput")
    onesd = nc.dram_tensor("onesd", [P, P], F32R, kind="ExternalInput")
    nident = nc.dram_tensor("nident", [P, P], F32R, kind="ExternalInput")
    out = nc.dram_tensor("out", [SQ, D], F32, kind="ExternalOutput")

    khT_d = nc.dram_tensor("khT_scr", [HK, SK], F32R)
    qhT_d = nc.dram_tensor("qhT_scr", [HK, SQ], F32R)
    vaug_d = nc.dram_tensor("vaug_scr", [SK, 8 * 130], F32R)
    rden_d = nc.dram_tensor("rden_scr", [H, SQ], F32)

    with tile.TileContext(nc) as tc:
        with tc.tile_pool(name="consts", bufs=1) as cst:
            ones_sb = cst.tile([P, P], F32R, tag="ones")
            nid_sb = cst.tile([P, P], F32R, tag="nid")
            bq_sb = cst.tile([P, 8], F32, tag="bq")
            bk_sb = cst.tile([P, 8], F32, tag="bk")
            bv_sb = cst.tile([1, HK], F32R, tag="bv")
            bo_sb = cst.tile([1, D], F32R, tag="bo")
            onecol = cst.tile([P, 16, 1], F32, tag="onecol")

            # ---------------- projections ----------------
            with (
                tc.tile_pool(name="wp", bufs=2) as wp,
                tc.tile_pool(name="xq", bufs=3) as xq,
                tc.tile_pool(name="pp", bufs=8, space="PSUM") as pp,
                tc.tile_pool(name="po", bufs=4) as po,
            ):
                # V projection -> vaug_d (startup phase)
                wv_sb = wp.tile([P, 8, HK], F32R, tag="w")
                vq0 = xq.tile([P, 8, 512], F32R, tag="xq")
                for _j in range(8):
                    nc.sync.dma_start(
                        vq0[:, _j],
                        vT.ap().rearrange("(j p) s -> p j s", p=P)[:, _j, ts(0, 512)],
                    )
                    nc.sync.dma_start(
                        wv_sb[:, _j],
                        wv.ap().rearrange("(j p) m -> p j m", p=P)[:, _j],
                    )
                nc.sync.dma_start(bq_sb[:], bq2.ap())
                nc.sync.dma_start(bk_sb[:], bk2.ap())
                nc.sync.dma_start(bv_sb[:], bvr.ap())
                nc.sync.dma_start(bo_sb[:], bor.ap())
                nc.sync.dma_start(ones_sb[:], onesd.ap())
                nc.sync.dma_start(nid_sb[:], nident.ap())
                nc.gpsimd.memset(onecol[:], 1.0)
                for c in range(4):
                    if c == 0:
                        vq = vq0
                    else:
                        vq = xq.tile([P, 8, 512], F32R, tag="xq")
                        for _j in range(8):
                            nc.sync.dma_start(
                                vq[:, _j],
                                vT.ap().rearrange("(j p) s -> p j s", p=P)[
                                    :, _j, ts(c, 512)
                                ],
                            )
                    vas = []
                    for blk in range(2):
                        # units u = (t_in, c2) pairs: blk0: t_in 0,1; blk1: 2,3
                        units = [(2 * blk + dt_, c2) for dt_ in range(2) for c2 in range(2)]
                        psvs = [pp.tile([P, 512], F32, tag="proj", name=f"psv{_u}") for _u in range(len(units))]
                        for j in range(8):
                            for u, (t_in, c2) in enumerate(units):
                                nc.tensor.matmul(
                                    psvs[u][:],
                                    vq[:, j, ts(t_in, P)],
                                    wv_sb[:, j, ts(c2, 512)],
                                    start=(j == 0),
                                    stop=False,
                                )
                        for u, (t_in, c2) in enumerate(units):
                            nc.tensor.matmul(
                                psvs[u][:],
                                ones_sb[0:1, 0:P],
                                bv_sb[:, ts(c2, 512)],
                                start=False,
                                stop=True,
                            )
                        for u, (t_in, c2) in enumerate(units):
                            if c2 == 0:
                                va = po.tile([P, 8, 130], F32R, tag="vaug")
                                vas.append((t_in, va))
                            else:
                                va = dict(vas)[t_in]
                            vag = va[:]
                            psv4 = psvs[u][:].rearrange(
                                "p (g h k) -> p g h k", g=4, h=2
                            )
                            nc.vector.tensor_copy(
                                vag[:, ds(4 * c2, 4), 0:130]
                                .rearrange("p g (h k) -> p g h k", h=2)[:, :, :, 0:64],
                                psv4,
                            )
                            if c2 == 1:
                                t = 4 * c + t_in
                                nc.vector.tensor_copy(
                                    vag[:, :, 64:130:65],
                                    onecol[:, :, 0].rearrange(
                                        "p (g h) -> p g h", h=2
                                    ),
                                )
                                nc.sync.dma_start(vaug_d.ap()[ts(t, P), :], va[:])
                        vas = [x for x in vas if x[0] >= 2 * blk + 2]

                # K projection -> khT_d
                wk_sb = wp.tile([P, 8, HK], F32R, tag="w")
                for _j in range(8):
                    nc.sync.dma_start(
                        wk_sb[:, _j],
                        wk.ap().rearrange("(j p) m -> p j m", p=P)[:, _j],
                    )
                for c in range(4):
                    kq = xq.tile([P, 8, 512], F32R, tag="xq")
                    for _j in range(8):
                        nc.sync.dma_start(
                            kq[:, _j],
                            kT.ap().rearrange("(j p) s -> p j s", p=P)[
                                :, _j, ts(c, 512)
                            ],
                        )
                    for iblk in range(2):
                        psks = [pp.tile([P, 512], F32, tag="proj", name=f"psk{_u}") for _u in range(4)]
                        for j in range(8):
                            for u in range(4):
                                nc.tensor.matmul(
                                    psks[u][:],
                                    wk_sb[:, j, ts(4 * iblk + u, P)],
                                    kq[:, j, :],
                                    start=(j == 0),
                                    stop=(j == 7),
                                )
                        for u in range(4):
                            i = 4 * iblk + u
                            ok = po.tile([P, 512], F32R, tag="projout")
                            nc.vector.tensor_scalar_add(
                                ok[:], psks[u][:], bk_sb[:, i : i + 1]
                            )
                            nc.sync.dma_start(khT_d.ap()[ts(i, P), ts(c, 512)], ok[:])

                # Q projection -> qhT_d
                wq_sb = wp.tile([P, 8, HK], F32R, tag="w")
                for _j in range(8):
                    nc.sync.dma_start(
                        wq_sb[:, _j],
                        wq.ap().rearrange("(j p) m -> p j m", p=P)[:, _j],
                    )
                for c in range(2):
                    qq = xq.tile([P, 8, 512], F32R, tag="xq")
                    for _j in range(8):
                        nc.sync.dma_start(
                            qq[:, _j],
                            qT.ap().rearrange("(j p) s -> p j s", p=P)[
                                :, _j, ts(c, 512)
                            ],
                        )
                    for iblk in range(2):
                        psqs = [pp.tile([P, 512], F32, tag="proj", name=f"psq{_u}") for _u in range(4)]
                        for j in range(8):
                            for u in range(4):
                                nc.tensor.matmul(
                                    psqs[u][:],
                                    wq_sb[:, j, ts(4 * iblk + u, P)],
                                    qq[:, j, :],
                                    start=(j == 0),
                                    stop=(j == 7),
                                )
                        for u in range(4):
                            i = 4 * iblk + u
                            oq = po.tile([P, 512], F32R, tag="projout")
                            nc.vector.tensor_scalar_add(
                                oq[:], psqs[u][:], bq_sb[:, i : i + 1]
                            )
                            nc.sync.dma_start(qhT_d.ap()[ts(i, P), ts(c, 512)], oq[:])

            # ---------------- attention ----------------
            with tc.tile_pool(name="aop", bufs=1) as aop:
              ao_sb = aop.tile([P, 8, SQ], F32R, tag="ao")
              with (
                tc.tile_pool(name="mcp", bufs=1) as mcp,
                  tc.tile_pool(name="khp", bufs=2) as khp,
                  tc.tile_pool(name="vap", bufs=2) as vap,
                  tc.tile_pool(name="qhp", bufs=2) as qhp,
                  tc.tile_pool(name="ep", bufs=2) as ep,
                  tc.tile_pool(name="mup", bufs=1) as mup,
                  tc.tile_pool(name="pss", bufs=2, space="PSUM") as pss,
                  tc.tile_pool(name="pso", bufs=1, space="PSUM") as pso,
                  tc.tile_pool(name="pcvp", bufs=1, space="PSUM") as pcvp,
                  tc.tile_pool(name="osb", bufs=2) as osb,
                  tc.tile_pool(name="cmp", bufs=1) as cmp,
                  tc.tile_pool(name="pcvs", bufs=1) as pcvs,
              ):
                  mc_sb = mcp.tile([P, 16, SQ], F32R, tag="mc")
                  for half in range(2):
                      mcu = mup.tile([P, 8, SQ], U8, tag="mcu")
                      nc.sync.dma_start(
                          mcu[:],
                          mcT.ap().rearrange("(t p) s -> p t s", p=P)[
                              :, ds(8 * half, 8), :
                          ],
                      )
                      for qtr in range(2):
                          nc.vector.tensor_copy(
                              mc_sb[:, ds(8 * half + 4 * qtr, 4), :],
                              mcu[:, ds(4 * qtr, 4), :],
                          )

                  # cmsum[sq] = sum_sk mc (shared denominator correction),
                  # broadcast to partition 64 via a ones K=1 matmul
                  ps_cm = pss.tile([1, SQ], F32, tag="pss")
                  for t in range(16):
                      for c2 in range(2):
                          nc.tensor.matmul(
                              ps_cm[:, ts(c2, 512)],
                              ones_sb[:, 0:1],
                              mc_sb[:, t, ts(c2, 512)],
                              start=(t == 0),
                              stop=(t == 15),
                          )
                  cmr = cmp.tile([1, SQ], F32R, tag="cm")
                  nc.vector.tensor_copy(cmr[:], ps_cm[:])
                  ps_cmb = pss.tile([65, SQ], F32, tag="pss")
                  for c2 in range(2):
                      nc.tensor.matmul(
                          ps_cmb[:, ts(c2, 512)],
                          ones_sb[0:1, 0:65],
                          cmr[:, ts(c2, 512)],
                          start=True,
                          stop=True,
                      )
                  cm_sb = cmp.tile([65, SQ], F32, tag="cm")
                  nc.vector.tensor_copy(cm_sb[:], ps_cmb[:])

                  for g in range(8):
                      khT_pair = khp.tile([P, SK], F32R, tag="kh")
                      nc.sync.dma_start(khT_pair[:], khT_d.ap()[ds(g * P, P), :])
                      vaug_pair = vap.tile([P, 16, 258], F32R, tag="va")
                      nc.sync.dma_start(
                          vaug_pair[:, :, 0:130],
                          vaug_d.ap().rearrange("(t p) c -> p t c", p=P)[
                              :, :, ds(g * 130, 130)
                          ],
                      )
                      for _hh in range(2):
                          nc.sync.dma_start(
                              vaug_pair[:, :, ds(130 + 64 * _hh, 64)],
                              vaug_d.ap().rearrange("(t p) c -> p t c", p=P)[
                                  :, :, ds(g * 130 + 65 * _hh, 64)
                              ],
                          )
                      qh_pair = qhp.tile([P, SQ], F32R, tag="qh")
                      nc.sync.dma_start(qh_pair[:], qhT_d.ap()[ds(g * P, P), :])
                      pcv = pcvp.tile([P, SQ], F32, tag="pcv")
                      pcv_sb = pcvs.tile([P, SQ], F32, tag="pcvsb")
                      for hh in range(2):
                          base = 64 * hh
                          ps_o = pso.tile([65, SQ], F32, tag="pso")
                          for t in range(16):
                              ps_s = pss.tile([P, SQ], F32, tag="pss")
                              for c2 in range(2):
                                  nc.tensor.matmul(
                                      ps_s[:, ts(c2, 512)],
                                      khT_pair[base : base + 64, ts(t, P)],
                                      qh_pair[base : base + 64, ts(c2, 512)],
                                      start=True,
                                      stop=False,
                                  )
                                  nc.tensor.matmul(
                                      ps_s[:, ts(c2, 512)],
                                      nid_sb[:],
                                      mc_sb[:, t, ts(c2, 512)],
                                      start=False,
                                      stop=True,
                                  )
                              e = ep.tile([P, SQ], F32R, tag="e")
                              nc.scalar.activation(e[:], ps_s[:], AF.Exp, scale=0.125)
                              for c2 in range(2):
                                  nc.tensor.matmul(
                                      ps_o[:, ts(c2, 512)],
                                      vaug_pair[:, t, ds(65 * hh, 65)],
                                      e[:, ts(c2, 512)],
                                      start=(t == 0),
                                      stop=False,
                                  )
                                  if hh == 0:
                                      # both heads' value corrections in one
                                      # M=128 matmul: rows 0:64 = head 2g,
                                      # rows 64:128 = head 2g+1
                                      nc.tensor.matmul(
                                          pcv[:, ts(c2, 512)],
                                          vaug_pair[:, t, 130:258],
                                          mc_sb[:, t, ts(c2, 512)],
                                          start=(t == 0),
                                          stop=(t == 15),
                                      )
                          if hh == 0:
                              nc.vector.tensor_copy(pcv_sb[:], pcv[:])
                          # normalize head 2g+hh
                          o_sb = osb.tile([65, SQ], F32, tag="osb")
                          nc.vector.tensor_tensor(
                              o_sb[0:64, :],
                              ps_o[0:64, :],
                              pcv_sb[base : base + 64, :],
                              ADD,
                          )
                          nc.vector.tensor_tensor(
                              o_sb[64:65, :], ps_o[64:65, :], cm_sb[64:65, :], ADD
                          )
                          nc.vector.reciprocal(o_sb[64:65, :], o_sb[64:65, :])
                          nc.sync.dma_start(rden_d.ap()[2 * g + hh : 2 * g + hh + 1, :], o_sb[64:65, :])
                          rbc = osb.tile([64, SQ], F32, tag="rbc")
                          nc.sync.dma_start(
                              rbc[:],
                              rden_d.ap()[2 * g + hh : 2 * g + hh + 1, :].to_broadcast((64, SQ)),
                          )
                          if hh == 0:
                              nc.vector.tensor_tensor(
                                  ao_sb[0:64, g, :], o_sb[0:64, :], rbc[:], MULT
                              )
                          else:
                              tmpn = osb.tile([64, SQ], F32R, tag="rbc")
                              nc.vector.tensor_tensor(
                                  tmpn[:], o_sb[0:64, :], rbc[:], MULT
                              )
                              nc.sync.dma_start(ao_sb[64:128, g, :], tmpn[:])

              # ---------------- output projection ----------------
              with (
                  tc.tile_pool(name="wop", bufs=1) as wop,
                  tc.tile_pool(name="pp2", bufs=8, space="PSUM") as pp2,
                  tc.tile_pool(name="po2", bufs=3) as po2,
              ):
                  wo_sb = wop.tile([P, 8, D], F32R, tag="wo")
                  for _j in range(8):
                      nc.sync.dma_start(
                          wo_sb[:, _j],
                          wo.ap().rearrange("(j p) m -> p j m", p=P)[:, _j],
                      )
                  scs = [(s, c) for s in range(8) for c in range(2)]
                  for blk in range(4):
                      units = scs[4 * blk : 4 * blk + 4]
                      psos = [pp2.tile([P, 512], F32, tag="op", name=f"pso2_{_u}") for _u in range(len(units))]
                      for j in range(8):
                          for u, (s, c) in enumerate(units):
                              nc.tensor.matmul(
                                  psos[u][:],
                                  ao_sb[:, j, ts(s, P)],
                                  wo_sb[:, j, ts(c, 512)],
                                  start=(j == 0),
                                  stop=False,
                              )
                      for u, (s, c) in enumerate(units):
                          nc.tensor.matmul(
                              psos[u][:],
                              ones_sb[0:1, 0:P],
                              bo_sb[:, ts(c, 512)],
                              start=False,
                              stop=True,
                          )
                          oo = po2.tile([P, 512], F32, tag="oo")
                          nc.vector.tensor_copy(oo[:], psos[u][:])
                          nc.sync.dma_start(out.ap()[ts(s, P), ts(c, 512)], oo[:])

    nc.compile()
    return nc


def make_host_inputs(q, k, v, mask, Wq, bq, Wk, bk, Wv, bv, Wo, bo):
    """Full inputs -> list of 8 per-core input dicts."""
    q = np.asarray(q, np.float32)
    k = np.asarray(k, np.float32)
    v = np.asarray(v, np.float32)
    mask = np.asarray(mask)
    r = round_f32r

    shared = {
        "wq": r(np.asarray(Wq, np.float32).transpose(1, 0, 2).reshape(D, HK)),
        "wk": r(np.asarray(Wk, np.float32).transpose(1, 0, 2).reshape(D, HK)),
        "wv": r(np.asarray(Wv, np.float32).transpose(1, 0, 2).reshape(D, HK)),
        "wo": r(np.asarray(Wo, np.float32)),
        "bq2": np.ascontiguousarray(
            np.asarray(bq, np.float32).reshape(HK).reshape(8, P).T
        ),
        "bk2": np.ascontiguousarray(
            np.asarray(bk, np.float32).reshape(HK).reshape(8, P).T
        ),
        "bvr": r(np.asarray(bv, np.float32).reshape(1, HK)),
        "bor": r(np.asarray(bo, np.float32).reshape(1, D)),
        "onesd": np.ones((P, P), np.float32),
        "nident": np.ascontiguousarray(-BIG * np.eye(P, dtype=np.float32)),
    }

    in_maps = []
    for core in range(8):
        b, j = divmod(core, 2)
        qs = q[b, j * SQ : (j + 1) * SQ, :]
        ms = mask[b, j * SQ : (j + 1) * SQ, :]
        m = dict(shared)
        m["qT"] = r(np.ascontiguousarray(qs.T))
        m["kT"] = r(np.ascontiguousarray(k[b].T))
        m["vT"] = r(np.ascontiguousarray(v[b].T))
        m["mcT"] = np.ascontiguousarray((~ms).T).astype(np.uint8)
        in_maps.append(m)
    return in_maps


def assemble_output(results):
    """8 per-core out [SQ, D] -> full [4, 2048, 1024]."""
    B, S = 4, 2048
    full = np.empty((B, S, D), np.float32)
    for core, res in enumerate(results):
        b, j = divmod(core, 2)
        full[b, j * SQ : (j + 1) * SQ, :] = res["out"]
    return full


class CompiledSpmd:
    def __init__(self, nc: bass.Bass, n_cores: int):
        bass2jax.install_neuronx_cc_hook()
        assert nc.dbg_addr is None, "build with debug=False"
        partition_name = (
            nc.partition_id_tensor.name if nc.partition_id_tensor else None
        )
        in_names, out_names, out_avals, zero_outs = [], [], [], []
        for alloc in nc.m.functions[0].allocations:
            if not isinstance(alloc, mybir.MemoryLocationSet):
                continue
            name = alloc.memorylocations[0].name
            if alloc.kind == "ExternalInput":
                if name != partition_name:
                    in_names.append(name)
            elif alloc.kind == "ExternalOutput":
                shape = tuple(alloc.tensor_shape)
                dtype = mybir.dt.np(alloc.dtype)
                out_names.append(name)
                out_avals.append(jax.core.ShapedArray(shape, dtype))
                zero_outs.append(np.zeros(shape, dtype))
        n_params = len(in_names)
        n_outs = len(out_avals)
        all_in_names = list(in_names) + list(out_names)
        if partition_name is not None:
            all_in_names.append(partition_name)

        def _body(*args):
            operands = list(args)
            if partition_name is not None:
                operands.append(bass2jax.partition_id_tensor())
            outs = bass2jax._bass_exec_p.bind(
                *operands,
                out_avals=tuple(out_avals),
                in_names=tuple(all_in_names),
                out_names=tuple(out_names),
                lowering_input_output_aliases=(),
                sim_require_finite=True,
                sim_require_nnan=True,
                nc=nc,
            )
            return tuple(outs)

        devices = jax.devices()[:n_cores]
        assert len(devices) == n_cores
        mesh = Mesh(np.asarray(devices), ("core",))
        self._mesh = mesh
        donate = tuple(range(n_params, n_params + n_outs))
        self._sharded = jax.jit(
            shard_map(
                _body,
                mesh=mesh,
                in_specs=(PartitionSpec("core"),) * (n_params + n_outs),
                out_specs=(PartitionSpec("core"),) * n_outs,
                check_rep=False,
            ),
            donate_argnums=donate,
            keep_unused=True,
        )
        self.in_names = in_names
        self.out_names = out_names
        self.out_avals = out_avals
        self.zero_outs = zero_outs
        self.n_cores = n_cores

    def _concat_inputs(self, in_maps):
        per_core = [[np.asarray(m[n]) for n in self.in_names] for m in in_maps]
        return [
            np.concatenate([per_core[c][i] for c in range(self.n_cores)], axis=0)
            for i in range(len(self.in_names))
        ]

    def run(self, in_maps, repeats: int = 1):
        """Returns (results_per_core, wall_times_s list of len repeats).

        Inputs and donated zero-output buffers are device_put outside the
        timed region, so wall time ~= dispatch + NEFF execution.
        """
        from jax.sharding import NamedSharding

        mesh = self._mesh
        shard = NamedSharding(mesh, PartitionSpec("core"))
        concat_in = [
            jax.device_put(a, shard) for a in self._concat_inputs(in_maps)
        ]
        rep_zeros = [
            [
                jax.device_put(
                    np.zeros((self.n_cores * z.shape[0], *z.shape[1:]), z.dtype),
                    shard,
                )
                for z in self.zero_outs
            ]
            for _ in range(repeats)
        ]
        jax.block_until_ready(concat_in)
        jax.block_until_ready(rep_zeros)
        times = []
        out_arrs = None
        for r in range(repeats):
            t0 = time.perf_counter()
            out_arrs = self._sharded(*concat_in, *rep_zeros[r])
            jax.block_until_ready(out_arrs)
            times.append(time.perf_counter() - t0)
        results = [
            {
                name: np.asarray(out_arrs[i]).reshape(
                    self.n_cores, *self.out_avals[i].shape
                )[c]
                for i, name in enumerate(self.out_names)
            }
            for c in range(self.n_cores)
        ]
        return results, times


_COMPILED = None


def _get_compiled():
    global _COMPILED
    if _COMPILED is None:
        nc = build_mha()
        _COMPILED = CompiledSpmd(nc, 8)
    return _COMPILED


def kernel(**inputs) -> np.ndarray:
    comp = _get_compiled()
    in_maps = make_host_inputs(**inputs)
    results, _ = comp.run(in_maps, repeats=1)
    return assemble_output(results)



# revision 36
# speedup vs baseline: 1.5107x; 1.5107x over previous
"""Self-contained Trainium2 Bass kernel for nn_MultiHeadAttention_68367289417808.

kernel(**inputs) takes FULL unsharded inputs (as in reference.setup_inputs())
and returns the FULL [4, 2048, 1024] output.

Sharding: 8 cores = (batch 4) x (query-half 2); no collectives needed.

Per-core pipeline (all matmuls bf16, PSUM fp32):
  - q/k/v/mask/weights SBUF-resident (no DRAM scratch roundtrips)
  - the reference's masked_fill(-1e-6) semantics give unnormalized weights
    u = m*exp(s) + (1-m).  We compute w = (exp(s)-1)*m on the vector engine
    (one 4x-mode scalar_tensor_tensor per tile) so u = w + 1, and fold the
    "+1" into a host-precomputed rank-1 correction: numerator += colsum(vh),
    denominator += SK.  No mask matmul, no mask@V correction on PE.
  - per-head-pair Q/K/V projections are interleaved into the previous pair's
    attention t-loop as PE filler work; attn@V trails scores by 2 t-steps.
  - output projection + bias; per-core [1024, 1024] slices assembled on host.
"""
import time
from collections import deque

import jax
import numpy as np
from jax.experimental.shard_map import shard_map
from jax.sharding import Mesh, PartitionSpec

import concourse.bass as bass
import concourse.bacc as bacc
import concourse.mybir as mybir
import concourse.tile as tile
from concourse import bass2jax
from concourse.bass import ts, ds

F32 = mybir.dt.float32
BF16 = mybir.dt.bfloat16
FP8 = mybir.dt.float8e4
BF16_NP = mybir.dt.np(mybir.dt.bfloat16)
FP8_NP = mybir.dt.np(mybir.dt.float8e4)
AF = mybir.ActivationFunctionType
MULT = mybir.AluOpType.mult
ADD = mybir.AluOpType.add

P = 128
SQ = 1024
SK = 2048
D = 1024
H = 16
DK = 64
HK = 1024


def build_mha():
    nc = bacc.Bacc("TRN2", target_bir_lowering=False)

    qT = nc.dram_tensor("qT", [D, SQ], BF16, kind="ExternalInput")
    kT = nc.dram_tensor("kT", [D, SK], BF16, kind="ExternalInput")
    vT = nc.dram_tensor("vT", [D, SK], BF16, kind="ExternalInput")
    mT = nc.dram_tensor("mT", [SK, SQ], FP8, kind="ExternalInput")
    wq = nc.dram_tensor("wq", [D, HK], BF16, kind="ExternalInput")
    wk = nc.dram_tensor("wk", [D, HK], BF16, kind="ExternalInput")
    wv = nc.dram_tensor("wv", [D, HK], BF16, kind="ExternalInput")
    wo = nc.dram_tensor("wo", [HK, D], BF16, kind="ExternalInput")
    bq2 = nc.dram_tensor("bq2", [P, 8], F32, kind="ExternalInput")
    bk2 = nc.dram_tensor("bk2", [P, 8], F32, kind="ExternalInput")
    bor = nc.dram_tensor("bor", [1, D], F32, kind="ExternalInput")
    csum_d = nc.dram_tensor("csum", [65, H], F32, kind="ExternalInput")
    out = nc.dram_tensor("out", [SQ, D], F32, kind="ExternalOutput")

    rden_d = nc.dram_tensor("rden_scr", [H, SQ], F32)

    with tile.TileContext(nc) as tc:
        with tc.tile_pool(name="consts", bufs=1) as cst:
            bq_sb = cst.tile([P, 8], F32, tag="bq")
            bk_sb = cst.tile([P, 8], F32, tag="bk")
            bo_bc = cst.tile([P, D], F32, tag="bo")
            csum_sb = cst.tile([65, H], F32, tag="cs")
            ones2 = cst.tile([P, H, 2], BF16, tag="on")

            nc.sync.dma_start(bq_sb[:], bq2.ap())
            nc.sync.dma_start(bk_sb[:], bk2.ap())
            nc.sync.dma_start(bo_bc[:], bor.ap().to_broadcast((P, D)))
            nc.sync.dma_start(csum_sb[:], csum_d.ap())
            nc.gpsimd.memset(ones2[:], 1.0)

            with tc.tile_pool(name="aop", bufs=1) as aop:
              ao_sb = aop.tile([P, 8, SQ], BF16, tag="ao")
              wo_c0 = aop.tile([P, 8, 512], BF16, tag="woc0")
              with tc.tile_pool(name="res", bufs=1) as res:
                vT_sb = res.tile([P, 8, SK], BF16, tag="vt")
                wv_sb = res.tile([P, 8, HK], BF16, tag="wv")
                kT_sb = res.tile([P, 8, SK], BF16, tag="kt")
                wk_sb = res.tile([P, 8, HK], BF16, tag="wk")
                m_sb = res.tile([P, H, SQ], FP8, tag="m")

                with tc.tile_pool(name="qh8", bufs=8) as qh8p:
                  qh8 = [qh8p.tile([P, SQ], BF16, tag="qh8", name=f"qh{_g}")
                         for _g in range(8)]
                  # Q projection upfront: needs only qT+wq (small DMA), runs
                  # while the big kT/vT/m loads stream in; qT/wq then free.
                  with (
                      tc.tile_pool(name="qtmp", bufs=1) as qtmp,
                      tc.tile_pool(name="qpj", bufs=2, space="PSUM") as qpj,
                  ):
                    qT_sb = qtmp.tile([P, 8, SQ], BF16, tag="qt")
                    wq_sb = qtmp.tile([P, 8, HK], BF16, tag="wq")
                    # DMA issue order matches PE consumption order:
                    # wq+qT(c0) -> Q chunks c0; qT(c1) -> Q chunks c1;
                    # kT/wk -> pair-0 K chunks; vT/wv -> V; mask last.
                    for _j in range(8):
                        nc.sync.dma_start(
                            wq_sb[:, _j],
                            wq.ap().rearrange("(j p) m -> p j m", p=P)[:, _j],
                        )
                    for half in range(2):
                        for _j in range(8):
                            nc.sync.dma_start(
                                qT_sb[:, _j, ts(half, 512)],
                                qT.ap().rearrange("(j p) s -> p j s", p=P)[
                                    :, _j, ts(half, 512)
                                ],
                            )
                    for _j in range(8):
                        nc.sync.dma_start(
                            wk_sb[:, _j],
                            wk.ap().rearrange("(j p) m -> p j m", p=P)[:, _j],
                        )
                    for half in range(2):
                        for _j in range(8):
                            nc.sync.dma_start(
                                kT_sb[:, _j, ts(half, SK // 2)],
                                kT.ap().rearrange("(j p) s -> p j s", p=P)[
                                    :, _j, ts(half, SK // 2)
                                ],
                            )
                    for _j in range(8):
                        nc.sync.dma_start(
                            wv_sb[:, _j],
                            wv.ap().rearrange("(j p) m -> p j m", p=P)[:, _j],
                        )
                    for half in range(2):
                        for _j in range(8):
                            nc.sync.dma_start(
                                vT_sb[:, _j, ts(half, SK // 2)],
                                vT.ap().rearrange("(j p) s -> p j s", p=P)[
                                    :, _j, ts(half, SK // 2)
                                ],
                            )
                    for c in range(4):
                        nc.sync.dma_start(
                            m_sb[:, ds(4 * c, 4), :],
                            mT.ap().rearrange("(t p) s -> p t s", p=P)[
                                :, ds(4 * c, 4), :
                            ],
                        )
                    for cc in range(2):
                        for gg in range(8):
                            psq = qpj.tile([P, 512], F32, tag="qj")
                            for j in range(8):
                                nc.tensor.matmul(
                                    psq[:],
                                    wq_sb[:, j, ds(P * gg, P)],
                                    qT_sb[:, j, ts(cc, 512)],
                                    start=(j == 0),
                                    stop=(j == 7),
                                )
                            nc.scalar.activation(
                                qh8[gg][:, ts(cc, 512)],
                                psq[:],
                                AF.Identity,
                                bias=bq_sb[:, gg : gg + 1],
                            )

                  with (
                    tc.tile_pool(name="khp", bufs=2) as khp,
                    tc.tile_pool(name="vap", bufs=2) as vap,
                    tc.tile_pool(name="ep", bufs=3) as ep,
                    tc.tile_pool(name="wpb", bufs=6) as wpb,
                    tc.tile_pool(name="osb", bufs=1) as osb,
                    tc.tile_pool(name="rbp", bufs=1) as rbp,
                    tc.tile_pool(name="tnp", bufs=1) as tnp,
                    tc.tile_pool(name="pss", bufs=2, space="PSUM") as pss,
                    tc.tile_pool(name="pso", bufs=1, space="PSUM") as pso,
                    tc.tile_pool(name="ppj", bufs=2, space="PSUM") as ppj,
                ):
                    pair_tiles = {}

                    def make_thunks(gg):
                        """Projection work for head pair gg as a list of
                        thunks, each ~8 matmuls + one vector op."""
                        kh = khp.tile([P, SK], BF16, tag="kh")
                        va = vap.tile([P, H, 130], BF16, tag="va")
                        pair_tiles[gg] = (kh, va)
                        thunks = []

                        def k_chunk(cc):
                            def run():
                                psk = ppj.tile([P, 512], F32, tag="pj")
                                for j in range(8):
                                    nc.tensor.matmul(
                                        psk[:],
                                        wk_sb[:, j, ds(P * gg, P)],
                                        kT_sb[:, j, ts(cc, 512)],
                                        start=(j == 0),
                                        stop=(j == 7),
                                    )
                                nc.scalar.activation(
                                    kh[:, ts(cc, 512)],
                                    psk[:],
                                    AF.Identity,
                                    bias=bk_sb[:, gg : gg + 1],
                                )
                            return run

                        def v_chunk(cc):
                            def run():
                                psv = ppj.tile([P, 512], F32, tag="pj")
                                for tt in range(4):
                                    t = 4 * cc + tt
                                    for j in range(8):
                                        nc.tensor.matmul(
                                            psv[:, ts(tt, P)],
                                            vT_sb[:, j, ts(t, P)],
                                            wv_sb[:, j, ds(P * gg, P)],
                                            start=(j == 0),
                                            stop=(j == 7),
                                        )
                                nc.scalar.activation(
                                    va[:, ds(4 * cc, 4), :].rearrange(
                                        "p t (h k) -> p t h k", h=2
                                    )[:, :, :, 0:64],
                                    psv[:].rearrange(
                                        "p (t h k) -> p t h k", t=4, h=2
                                    ),
                                    AF.Copy,
                                )
                            return run

                        def v_ones():
                            nc.vector.tensor_copy(va[:, :, 64:130:65], ones2[:])

                        for cc in range(4):
                            thunks.append(k_chunk(cc))
                        for cc in range(4):
                            thunks.append(v_chunk(cc))
                        thunks.append(v_ones)
                        return thunks

                    pending = deque(make_thunks(0))
                    while pending:
                        pending.popleft()()

                    cur = {}
                    pso_tiles = {}

                    def attnv(g_, hh_, t_, w_):
                        va_ = cur[g_][1]
                        ps_o_ = pso_tiles[(g_, hh_)]
                        for c2 in range(2):
                            nc.tensor.matmul(
                                ps_o_[:, ts(c2, 512)],
                                va_[:, t_, ds(65 * hh_, 65)],
                                w_[:, ts(c2, 512)],
                                start=(t_ == 0),
                                stop=(t_ == 15),
                            )

                    def normalize(g_, hh_):
                        # numerator += colsum(vh); denominator = row 64
                        # (+SK via csum row 64); ao = numerator/denominator
                        h_ = 2 * g_ + hh_
                        ps_o_ = pso_tiles.pop((g_, hh_))
                        o_sb = osb.tile([65, SQ], F32, tag="osb")
                        nc.scalar.activation(
                            o_sb[:],
                            ps_o_[:],
                            AF.Identity,
                            bias=csum_sb[:, h_ : h_ + 1],
                        )
                        nc.vector.reciprocal(o_sb[64:65, :], o_sb[64:65, :])
                        nc.sync.dma_start(
                            rden_d.ap()[h_ : h_ + 1, :], o_sb[64:65, :]
                        )
                        rbc = rbp.tile([64, SQ], F32, tag="rbc")
                        nc.sync.dma_start(
                            rbc[:],
                            rden_d.ap()[h_ : h_ + 1, :].to_broadcast((64, SQ)),
                        )
                        if hh_ == 0:
                            nc.gpsimd.tensor_tensor(
                                ao_sb[0:64, g_, :], o_sb[0:64, :], rbc[:], MULT
                            )
                        else:
                            tmpn = tnp.tile([64, SQ], BF16, tag="tn")
                            nc.gpsimd.tensor_tensor(
                                tmpn[:], o_sb[0:64, :], rbc[:], MULT
                            )
                            nc.sync.dma_start(ao_sb[64:128, g_, :], tmpn[:])

                    def drain_one(hist):
                        g_, hh_, t_, w_ = hist.pop(0)
                        attnv(g_, hh_, t_, w_)
                        if t_ == 15:
                            normalize(g_, hh_)

                    def maybe_drain(hist, g, hh, t):
                        # drain prev-(g,hh) items promptly, but defer the
                        # current accumulator's first attn@V until the
                        # single pso slot is freed by the previous
                        # normalize (~slot 3)
                        if len(hist) <= 3:
                            return
                        g_, hh_, t_, _ = hist[0]
                        if (g_, hh_) == (g, hh) and t < 4:
                            return
                        drain_one(hist)

                    # one continuous software-pipelined stream over all
                    # (pair, head, t) steps; attn@V trails scores by 3 steps
                    hist = []
                    slot = 0
                    for g in range(8):
                        if g == 7:
                            # prefetch first half of Wo during the last pair
                            for _j in range(8):
                                nc.sync.dma_start(
                                    wo_c0[:, _j],
                                    wo.ap().rearrange("(j p) m -> p j m", p=P)[
                                        :, _j, 0:512
                                    ],
                                )
                        if g < 7:
                            pending.extend(make_thunks(g + 1))
                        cur[g] = pair_tiles.pop(g)
                        kh, va = cur[g]
                        qh = qh8[g]
                        for hh in range(2):
                            base = 64 * hh
                            pso_tiles[(g, hh)] = pso.tile(
                                [65, SQ], F32, tag="pso", name=f"pso_{g}_{hh}"
                            )
                            for t in range(16):
                                ps_s = pss.tile([P, SQ], F32, tag="pss")
                                for c2 in range(2):
                                    nc.tensor.matmul(
                                        ps_s[:, ts(c2, 512)],
                                        kh[base : base + 64, ts(t, P)],
                                        qh[base : base + 64, ts(c2, 512)],
                                        start=True,
                                        stop=True,
                                    )
                                e = ep.tile([P, SQ], BF16, tag="e")
                                nc.scalar.activation(
                                    e[:], ps_s[:], AF.Exp, scale=0.125
                                )
                                # w = (e-1)*m  (masked -> 0; the "+1" is the
                                # rank-1 csum correction applied at normalize)
                                w = wpb.tile([P, SQ], BF16, tag="w")
                                nc.vector.scalar_tensor_tensor(
                                    w[:], e[:], -1.0, m_sb[:, t, :], ADD, MULT
                                )
                                hist.append((g, hh, t, w))
                                maybe_drain(hist, g, hh, t)
                                if slot % 3 == 1 and pending:
                                    pending.popleft()()
                                slot += 1
                        cur.pop(g - 1, None)
                    while hist:
                        drain_one(hist)

              # ---------------- output projection ----------------
              with (
                  tc.tile_pool(name="wop", bufs=1) as wop,
                  tc.tile_pool(name="pp2", bufs=8, space="PSUM") as pp2,
                  tc.tile_pool(name="po2", bufs=3) as po2,
              ):
                  wo_c1 = wop.tile([P, 8, 512], BF16, tag="woc1")
                  for _j in range(8):
                      nc.sync.dma_start(
                          wo_c1[:, _j],
                          wo.ap().rearrange("(j p) m -> p j m", p=P)[
                              :, _j, 512:1024
                          ],
                      )
                  # all c=0 units first (prefetched half), c=1 half streams in
                  scs = [(s, c) for c in range(2) for s in range(8)]
                  for blk in range(4):
                      units = scs[4 * blk : 4 * blk + 4]
                      psos = [
                          pp2.tile([P, 512], F32, tag="op", name=f"pso2_{_u}")
                          for _u in range(len(units))
                      ]
                      for j in range(8):
                          for u, (s, c) in enumerate(units):
                              nc.tensor.matmul(
                                  psos[u][:],
                                  ao_sb[:, j, ts(s, P)],
                                  (wo_c0 if c == 0 else wo_c1)[:, j, :],
                                  start=(j == 0),
                                  stop=(j == 7),
                              )
                      for u, (s, c) in enumerate(units):
                          oo = po2.tile([P, 512], F32, tag="oo")
                          nc.vector.tensor_tensor(
                              oo[:], psos[u][:], bo_bc[:, ts(c, 512)], ADD
                          )
                          nc.sync.dma_start(out.ap()[ts(s, P), ts(c, 512)], oo[:])

    nc.compile()
    return nc


def make_host_inputs(q, k, v, mask, Wq, bq, Wk, bk, Wv, bv, Wo, bo):
    """Full inputs -> list of 8 per-core input dicts."""
    q = np.asarray(q, np.float32)
    k = np.asarray(k, np.float32)
    v = np.asarray(v, np.float32)
    mask = np.asarray(mask)
    Wq = np.asarray(Wq, np.float32)
    Wk = np.asarray(Wk, np.float32)
    Wv = np.asarray(Wv, np.float32)
    Wo = np.asarray(Wo, np.float32)
    bq = np.asarray(bq, np.float32)
    bk = np.asarray(bk, np.float32)
    bv = np.asarray(bv, np.float32)
    bo = np.asarray(bo, np.float32)

    def b16(a):
        return np.ascontiguousarray(a).astype(BF16_NP)

    # bv contributes exactly bv[hk] to each normalized attention output
    # (weights sum to den), so its effect folds into the output bias.
    bo_eff = bo + bv.reshape(HK) @ Wo
    shared = {
        "wq": b16(Wq.transpose(1, 0, 2).reshape(D, HK)),
        "wk": b16(Wk.transpose(1, 0, 2).reshape(D, HK)),
        "wv": b16(Wv.transpose(1, 0, 2).reshape(D, HK)),
        "wo": b16(Wo),
        "bq2": np.ascontiguousarray(bq.reshape(HK).reshape(8, P).T),
        "bk2": np.ascontiguousarray(bk.reshape(HK).reshape(8, P).T),
        "bor": np.ascontiguousarray(bo_eff.reshape(1, D)),
    }

    in_maps = []
    for core in range(8):
        b, j = divmod(core, 2)
        qs = q[b, j * SQ : (j + 1) * SQ, :]
        ms = mask[b, j * SQ : (j + 1) * SQ, :]
        # rank-1 correction: colsum of vh per head; row 64 = +SK on denom
        vsum = v[b].sum(axis=0)  # [D]
        cs = np.einsum("d,hdk->hk", vsum, Wv)  # [H, DK] (vh excludes bv)
        csum = np.empty((65, H), np.float32)
        csum[0:64, :] = cs.T
        csum[64, :] = float(SK)
        m = dict(shared)
        m["qT"] = b16(qs.T)
        m["kT"] = b16(k[b].T)
        m["vT"] = b16(v[b].T)
        m["mT"] = np.ascontiguousarray(ms.T).astype(np.float32).astype(FP8_NP)
        m["csum"] = np.ascontiguousarray(csum)
        in_maps.append(m)
    return in_maps


def assemble_output(results):
    """8 per-core out [SQ, D] -> full [4, 2048, 1024]."""
    B, S = 4, 2048
    full = np.empty((B, S, D), np.float32)
    for core, res in enumerate(results):
        b, j = divmod(core, 2)
        full[b, j * SQ : (j + 1) * SQ, :] = res["out"]
    return full


class CompiledSpmd:
    def __init__(self, nc: bass.Bass, n_cores: int):
        bass2jax.install_neuronx_cc_hook()
        assert nc.dbg_addr is None, "build with debug=False"
        partition_name = (
            nc.partition_id_tensor.name if nc.partition_id_tensor else None
        )
        in_names, out_names, out_avals, zero_outs = [], [], [], []
        for alloc in nc.m.functions[0].allocations:
            if not isinstance(alloc, mybir.MemoryLocationSet):
                continue
            name = alloc.memorylocations[0].name
            if alloc.kind == "ExternalInput":
                if name != partition_name:
                    in_names.append(name)
            elif alloc.kind == "ExternalOutput":
                shape = tuple(alloc.tensor_shape)
                dtype = mybir.dt.np(alloc.dtype)
                out_names.append(name)
                out_avals.append(jax.core.ShapedArray(shape, dtype))
                zero_outs.append(np.zeros(shape, dtype))
        n_params = len(in_names)
        n_outs = len(out_avals)
        all_in_names = list(in_names) + list(out_names)
        if partition_name is not None:
            all_in_names.append(partition_name)

        def _body(*args):
            operands = list(args)
            if partition_name is not None:
                operands.append(bass2jax.partition_id_tensor())
            outs = bass2jax._bass_exec_p.bind(
                *operands,
                out_avals=tuple(out_avals),
                in_names=tuple(all_in_names),
                out_names=tuple(out_names),
                lowering_input_output_aliases=(),
                sim_require_finite=True,
                sim_require_nnan=True,
                nc=nc,
            )
            return tuple(outs)

        devices = jax.devices()[:n_cores]
        assert len(devices) == n_cores
        mesh = Mesh(np.asarray(devices), ("core",))
        self._mesh = mesh
        donate = tuple(range(n_params, n_params + n_outs))
        self._sharded = jax.jit(
            shard_map(
                _body,
                mesh=mesh,
                in_specs=(PartitionSpec("core"),) * (n_params + n_outs),
                out_specs=(PartitionSpec("core"),) * n_outs,
                check_rep=False,
            ),
            donate_argnums=donate,
            keep_unused=True,
        )
        self.in_names = in_names
        self.out_names = out_names
        self.out_avals = out_avals
        self.zero_outs = zero_outs
        self.n_cores = n_cores

    def _concat_inputs(self, in_maps):
        per_core = [[np.asarray(m[n]) for n in self.in_names] for m in in_maps]
        return [
            np.concatenate([per_core[c][i] for c in range(self.n_cores)], axis=0)
            for i in range(len(self.in_names))
        ]

    def run(self, in_maps, repeats: int = 1):
        """Returns (results_per_core, wall_times_s list of len repeats).

        Inputs and donated zero-output buffers are device_put outside the
        timed region, so wall time ~= dispatch + NEFF execution.
        """
        from jax.sharding import NamedSharding

        mesh = self._mesh
        shard = NamedSharding(mesh, PartitionSpec("core"))
        concat_in = [
            jax.device_put(a, shard) for a in self._concat_inputs(in_maps)
        ]
        rep_zeros = [
            [
                jax.device_put(
                    np.zeros((self.n_cores * z.shape[0], *z.shape[1:]), z.dtype),
                    shard,
                )
                for z in self.zero_outs
            ]
            for _ in range(repeats)
        ]
        jax.block_until_ready(concat_in)
        jax.block_until_ready(rep_zeros)
        times = []
        out_arrs = None
        for r in range(repeats):
            t0 = time.perf_counter()
            out_arrs = self._sharded(*concat_in, *rep_zeros[r])
            jax.block_until_ready(out_arrs)
            times.append(time.perf_counter() - t0)
        results = [
            {
                name: np.asarray(out_arrs[i]).reshape(
                    self.n_cores, *self.out_avals[i].shape
                )[c]
                for i, name in enumerate(self.out_names)
            }
            for c in range(self.n_cores)
        ]
        return results, times


_COMPILED = None


def _get_compiled():
    global _COMPILED
    if _COMPILED is None:
        nc = build_mha()
        _COMPILED = CompiledSpmd(nc, 8)
    return _COMPILED


def kernel(**inputs) -> np.ndarray:
    comp = _get_compiled()
    in_maps = make_host_inputs(**inputs)
    results, _ = comp.run(in_maps, repeats=1)
    return assemble_output(results)


# revision 50
# speedup vs baseline: 1.5193x; 1.0057x over previous
"""Self-contained Trainium2 Bass kernel for nn_MultiHeadAttention_68367289417808.

kernel(**inputs) takes FULL unsharded inputs (as in reference.setup_inputs())
and returns the FULL [4, 2048, 1024] output.

Sharding: 8 cores = (batch 4) x (query-half 2); no collectives needed.

Per-core pipeline (all matmuls bf16, PSUM fp32):
  - q/k/v/mask/weights SBUF-resident (no DRAM scratch roundtrips)
  - the reference's masked_fill(-1e-6) semantics give unnormalized weights
    u = m*exp(s) + (1-m).  We compute w = (exp(s)-1)*m on the vector engine
    (one scalar_tensor_tensor per tile, mask stored as fp8 0/1) so u = w + 1,
    and fold the "+1" into a host-precomputed rank-1 correction:
    numerator += colsum(v@Wv), denominator += SK.  No mask matmul and no
    mask@V correction on PE; bv's effect folds into the output bias on host.
  - Q projection runs upfront under the input DMA shadow; per-head-pair K/V
    projections interleave into the previous pair's attention t-loop as PE
    filler; attn@V trails scores by ~4 steps in one continuous software
    pipeline across all (pair, head, t) steps; exp on Act, softmax
    normalization spread across DVE/Act/GPSIMD.
  - output projection + bias; per-core [1024, 1024] slices assembled on host.
"""
import time
from collections import deque

import jax
import numpy as np
from jax.experimental.shard_map import shard_map
from jax.sharding import Mesh, PartitionSpec

import concourse.bass as bass
import concourse.bacc as bacc
import concourse.mybir as mybir
import concourse.tile as tile
from concourse import bass2jax
from concourse.bass import ts, ds

F32 = mybir.dt.float32
BF16 = mybir.dt.bfloat16
FP8 = mybir.dt.float8e4
BF16_NP = mybir.dt.np(mybir.dt.bfloat16)
FP8_NP = mybir.dt.np(mybir.dt.float8e4)
AF = mybir.ActivationFunctionType
MULT = mybir.AluOpType.mult
ADD = mybir.AluOpType.add

P = 128
SQ = 1024
SK = 2048
D = 1024
H = 16
DK = 64
HK = 1024


def build_mha():
    nc = bacc.Bacc("TRN2", target_bir_lowering=False)

    qT = nc.dram_tensor("qT", [D, SQ], BF16, kind="ExternalInput")
    kT = nc.dram_tensor("kT", [D, SK], BF16, kind="ExternalInput")
    vT = nc.dram_tensor("vT", [D, SK], BF16, kind="ExternalInput")
    mT = nc.dram_tensor("mT", [SK, SQ], FP8, kind="ExternalInput")
    wq = nc.dram_tensor("wq", [D, HK], BF16, kind="ExternalInput")
    wk = nc.dram_tensor("wk", [D, HK], BF16, kind="ExternalInput")
    wv = nc.dram_tensor("wv", [D, HK], BF16, kind="ExternalInput")
    wo = nc.dram_tensor("wo", [HK, D], BF16, kind="ExternalInput")
    bq2 = nc.dram_tensor("bq2", [P, 8], F32, kind="ExternalInput")
    bk2 = nc.dram_tensor("bk2", [P, 8], F32, kind="ExternalInput")
    bor = nc.dram_tensor("bor", [1, D], F32, kind="ExternalInput")
    csum_d = nc.dram_tensor("csum", [65, H], F32, kind="ExternalInput")
    out = nc.dram_tensor("out", [SQ, D], F32, kind="ExternalOutput")

    rden_d = nc.dram_tensor("rden_scr", [H, SQ], F32)

    with tile.TileContext(nc) as tc:
        with tc.tile_pool(name="consts", bufs=1) as cst:
            bq_sb = cst.tile([P, 8], F32, tag="bq")
            bk_sb = cst.tile([P, 8], F32, tag="bk")
            bo_bc = cst.tile([P, D], F32, tag="bo")
            csum_sb = cst.tile([65, H], F32, tag="cs")
            ones2 = cst.tile([P, H, 2], BF16, tag="on")

            nc.sync.dma_start(bq_sb[:], bq2.ap())
            nc.sync.dma_start(bk_sb[:], bk2.ap())
            nc.sync.dma_start(bo_bc[:], bor.ap().to_broadcast((P, D)))
            nc.sync.dma_start(csum_sb[:], csum_d.ap())
            nc.gpsimd.memset(ones2[:], 1.0)

            with tc.tile_pool(name="aop", bufs=1) as aop:
              ao_sb = aop.tile([P, 8, SQ], BF16, tag="ao")
              wo_c0 = aop.tile([P, 8, 512], BF16, tag="woc0")
              with tc.tile_pool(name="res", bufs=1) as res:
                vT_sb = res.tile([P, 8, SK], BF16, tag="vt")
                wv_sb = res.tile([P, 8, HK], BF16, tag="wv")
                kT_sb = res.tile([P, 8, SK], BF16, tag="kt")
                wk_sb = res.tile([P, 8, HK], BF16, tag="wk")
                m_sb = res.tile([P, H, SQ], FP8, tag="m")

                with tc.tile_pool(name="qh8", bufs=8) as qh8p:
                  qh8 = [qh8p.tile([P, SQ], BF16, tag="qh8", name=f"qh{_g}")
                         for _g in range(8)]
                  # Q projection upfront: needs only qT+wq (small DMA), runs
                  # while the big kT/vT/m loads stream in; qT/wq then free.
                  with (
                      tc.tile_pool(name="qtmp", bufs=1) as qtmp,
                      tc.tile_pool(name="qpj", bufs=2, space="PSUM") as qpj,
                  ):
                    qT_sb = qtmp.tile([P, 8, SQ], BF16, tag="qt")
                    wq_sb = qtmp.tile([P, 8, HK], BF16, tag="wq")
                    # DMA issue order matches PE consumption order:
                    # qT(c0)+wq(first pairs) -> Q chunks c0; qT(c1) -> c1;
                    # kT/wk -> pair-0 K chunks; vT/wv -> V; mask last.
                    for _j in range(8):
                        nc.sync.dma_start(
                            qT_sb[:, _j, ts(0, 512)],
                            qT.ap().rearrange("(j p) s -> p j s", p=P)[
                                :, _j, ts(0, 512)
                            ],
                        )
                        nc.sync.dma_start(
                            wq_sb[:, _j],
                            wq.ap().rearrange("(j p) m -> p j m", p=P)[:, _j],
                        )
                    for _j in range(8):
                        nc.sync.dma_start(
                            qT_sb[:, _j, ts(1, 512)],
                            qT.ap().rearrange("(j p) s -> p j s", p=P)[
                                :, _j, ts(1, 512)
                            ],
                        )
                    for _j in range(8):
                        nc.sync.dma_start(
                            wk_sb[:, _j],
                            wk.ap().rearrange("(j p) m -> p j m", p=P)[:, _j],
                        )
                    for half in range(2):
                        for _j in range(8):
                            nc.sync.dma_start(
                                kT_sb[:, _j, ts(half, SK // 2)],
                                kT.ap().rearrange("(j p) s -> p j s", p=P)[
                                    :, _j, ts(half, SK // 2)
                                ],
                            )
                    nc.sync.dma_start(
                        m_sb[:, ds(0, 4), :],
                        mT.ap().rearrange("(t p) s -> p t s", p=P)[:, ds(0, 4), :],
                    )
                    for _j in range(8):
                        nc.sync.dma_start(
                            wv_sb[:, _j],
                            wv.ap().rearrange("(j p) m -> p j m", p=P)[:, _j],
                        )
                    for half in range(2):
                        for _j in range(8):
                            nc.sync.dma_start(
                                vT_sb[:, _j, ts(half, SK // 2)],
                                vT.ap().rearrange("(j p) s -> p j s", p=P)[
                                    :, _j, ts(half, SK // 2)
                                ],
                            )
                        if half == 0:
                            nc.sync.dma_start(
                                m_sb[:, ds(4, 4), :],
                                mT.ap().rearrange("(t p) s -> p t s", p=P)[
                                    :, ds(4, 4), :
                                ],
                            )
                    for c in range(2, 4):
                        nc.sync.dma_start(
                            m_sb[:, ds(4 * c, 4), :],
                            mT.ap().rearrange("(t p) s -> p t s", p=P)[
                                :, ds(4 * c, 4), :
                            ],
                        )
                    for cc in range(2):
                        for gg in range(8):
                            psq = qpj.tile([P, 512], F32, tag="qj")
                            for j in range(8):
                                nc.tensor.matmul(
                                    psq[:],
                                    wq_sb[:, j, ds(P * gg, P)],
                                    qT_sb[:, j, ts(cc, 512)],
                                    start=(j == 0),
                                    stop=(j == 7),
                                )
                            nc.scalar.activation(
                                qh8[gg][:, ts(cc, 512)],
                                psq[:],
                                AF.Identity,
                                bias=bq_sb[:, gg : gg + 1],
                            )

                  with (
                    tc.tile_pool(name="khp", bufs=2) as khp,
                    tc.tile_pool(name="vap", bufs=2) as vap,
                    tc.tile_pool(name="ep", bufs=4) as ep,
                    tc.tile_pool(name="wpb", bufs=7) as wpb,
                    tc.tile_pool(name="osb", bufs=1) as osb,
                    tc.tile_pool(name="rbp", bufs=1) as rbp,
                    tc.tile_pool(name="tnp", bufs=1) as tnp,
                    tc.tile_pool(name="pss", bufs=2, space="PSUM") as pss,
                    tc.tile_pool(name="pso", bufs=1, space="PSUM") as pso,
                    tc.tile_pool(name="ppj", bufs=2, space="PSUM") as ppj,
                ):
                    pair_tiles = {}

                    def make_thunks(gg):
                        """Projection work for head pair gg as a list of
                        thunks, each ~8 matmuls + one vector op."""
                        kh = khp.tile([P, SK], BF16, tag="kh")
                        va = vap.tile([P, H, 130], BF16, tag="va")
                        pair_tiles[gg] = (kh, va)
                        thunks = []

                        def k_chunk(cc):
                            def run():
                                psk = ppj.tile([P, 512], F32, tag="pj")
                                for j in range(8):
                                    nc.tensor.matmul(
                                        psk[:],
                                        wk_sb[:, j, ds(P * gg, P)],
                                        kT_sb[:, j, ts(cc, 512)],
                                        start=(j == 0),
                                        stop=(j == 7),
                                    )
                                nc.scalar.activation(
                                    kh[:, ts(cc, 512)],
                                    psk[:],
                                    AF.Identity,
                                    bias=bk_sb[:, gg : gg + 1],
                                )
                            return run

                        def v_chunk(cc):
                            def run():
                                psv = ppj.tile([P, 512], F32, tag="pj")
                                for tt in range(4):
                                    t = 4 * cc + tt
                                    for j in range(8):
                                        nc.tensor.matmul(
                                            psv[:, ts(tt, P)],
                                            vT_sb[:, j, ts(t, P)],
                                            wv_sb[:, j, ds(P * gg, P)],
                                            start=(j == 0),
                                            stop=(j == 7),
                                        )
                                nc.vector.tensor_copy(
                                    va[:, ds(4 * cc, 4), :].rearrange(
                                        "p t (h k) -> p t h k", h=2
                                    )[:, :, :, 0:64],
                                    psv[:].rearrange(
                                        "p (t h k) -> p t h k", t=4, h=2
                                    ),
                                )
                            return run

                        def v_ones():
                            nc.vector.tensor_copy(va[:, :, 64:130:65], ones2[:])

                        for cc in range(4):
                            thunks.append(k_chunk(cc))
                        for cc in range(4):
                            thunks.append(v_chunk(cc))
                        thunks.append(v_ones)
                        return thunks

                    pending = deque(make_thunks(0))
                    while pending:
                        pending.popleft()()

                    cur = {}
                    pso_tiles = {}

                    def attnv(g_, hh_, t_, w_):
                        va_ = cur[g_][1]
                        ps_o_ = pso_tiles[(g_, hh_)]
                        for c2 in range(2):
                            nc.tensor.matmul(
                                ps_o_[:, ts(c2, 512)],
                                va_[:, t_, ds(65 * hh_, 65)],
                                w_[:, ts(c2, 512)],
                                start=(t_ == 0),
                                stop=(t_ == 15),
                            )

                    def normalize(g_, hh_):
                        # numerator += colsum(vh); denominator = row 64
                        # (+SK via csum row 64); ao = numerator/denominator
                        h_ = 2 * g_ + hh_
                        ps_o_ = pso_tiles.pop((g_, hh_))
                        o_sb = osb.tile([65, SQ], F32, tag="osb")
                        nc.scalar.activation(
                            o_sb[:],
                            ps_o_[:],
                            AF.Identity,
                            bias=csum_sb[:, h_ : h_ + 1],
                        )
                        nc.vector.reciprocal(o_sb[64:65, :], o_sb[64:65, :])
                        nc.sync.dma_start(
                            rden_d.ap()[h_ : h_ + 1, :], o_sb[64:65, :]
                        )
                        rbc = rbp.tile([64, SQ], F32, tag="rbc")
                        nc.sync.dma_start(
                            rbc[:],
                            rden_d.ap()[h_ : h_ + 1, :].to_broadcast((64, SQ)),
                        )
                        if hh_ == 0:
                            nc.gpsimd.tensor_tensor(
                                ao_sb[0:64, g_, :], o_sb[0:64, :], rbc[:], MULT
                            )
                        else:
                            tmpn = tnp.tile([64, SQ], BF16, tag="tn")
                            nc.gpsimd.tensor_tensor(
                                tmpn[:], o_sb[0:64, :], rbc[:], MULT
                            )
                            nc.sync.dma_start(ao_sb[64:128, g_, :], tmpn[:])

                    def drain_one(hist):
                        g_, hh_, t_, w_ = hist.pop(0)
                        attnv(g_, hh_, t_, w_)
                        if t_ == 15:
                            normalize(g_, hh_)

                    def maybe_drain(hist, g, hh, t):
                        # drain prev-(g,hh) items promptly, but defer the
                        # current accumulator's first attn@V until the
                        # single pso slot is freed by the previous
                        # normalize (~slot 3)
                        if len(hist) <= 4:
                            return
                        g_, hh_, t_, _ = hist[0]
                        if (g_, hh_) == (g, hh) and t < 5:
                            return
                        drain_one(hist)

                    # one continuous software-pipelined stream over all
                    # (pair, head, t) steps; attn@V trails scores by 3 steps
                    hist = []
                    slot = 0
                    for g in range(8):
                        if g == 7:
                            # prefetch first half of Wo during the last pair
                            for _j in range(8):
                                nc.sync.dma_start(
                                    wo_c0[:, _j],
                                    wo.ap().rearrange("(j p) m -> p j m", p=P)[
                                        :, _j, 0:512
                                    ],
                                )
                        if g < 7:
                            pending.extend(make_thunks(g + 1))
                        cur[g] = pair_tiles.pop(g)
                        kh, va = cur[g]
                        qh = qh8[g]
                        # hh=1 first: its normalize ends in a partition-shift
                        # DMA; keeping the direct-write head (hh=0) last
                        # shortens the tail into the output projection
                        for hh in (1, 0):
                            base = 64 * hh
                            pso_tiles[(g, hh)] = pso.tile(
                                [65, SQ], F32, tag="pso", name=f"pso_{g}_{hh}"
                            )
                            for t in range(16):
                                ps_s = pss.tile([P, SQ], F32, tag="pss")
                                for c2 in range(2):
                                    nc.tensor.matmul(
                                        ps_s[:, ts(c2, 512)],
                                        kh[base : base + 64, ts(t, P)],
                                        qh[base : base + 64, ts(c2, 512)],
                                        start=True,
                                        stop=True,
                                    )
                                e = ep.tile([P, SQ], BF16, tag="e")
                                nc.scalar.activation(
                                    e[:], ps_s[:], AF.Exp, scale=0.125
                                )
                                # w = (e-1)*m  (masked -> 0; the "+1" is the
                                # rank-1 csum correction applied at normalize)
                                w = wpb.tile([P, SQ], BF16, tag="w")
                                nc.vector.scalar_tensor_tensor(
                                    w[:], e[:], -1.0, m_sb[:, t, :], ADD, MULT
                                )
                                hist.append((g, hh, t, w))
                                maybe_drain(hist, g, hh, t)
                                if slot % 3 == 1 and pending:
                                    pending.popleft()()
                                slot += 1
                        cur.pop(g - 1, None)
                    while hist:
                        drain_one(hist)

              # ---------------- output projection ----------------
              with (
                  tc.tile_pool(name="wop", bufs=1) as wop,
                  tc.tile_pool(name="pp2", bufs=8, space="PSUM") as pp2,
                  tc.tile_pool(name="po2", bufs=3) as po2,
              ):
                  wo_c1 = wop.tile([P, 8, 512], BF16, tag="woc1")
                  for _j in range(8):
                      nc.sync.dma_start(
                          wo_c1[:, _j],
                          wo.ap().rearrange("(j p) m -> p j m", p=P)[
                              :, _j, 512:1024
                          ],
                      )
                  # all c=0 units first (prefetched half), c=1 half streams in
                  scs = [(s, c) for c in range(2) for s in range(8)]
                  for blk in range(4):
                      units = scs[4 * blk : 4 * blk + 4]
                      psos = [
                          pp2.tile([P, 512], F32, tag="op", name=f"pso2_{_u}")
                          for _u in range(len(units))
                      ]
                      for j in range(8):
                          for u, (s, c) in enumerate(units):
                              nc.tensor.matmul(
                                  psos[u][:],
                                  ao_sb[:, j, ts(s, P)],
                                  (wo_c0 if c == 0 else wo_c1)[:, j, :],
                                  start=(j == 0),
                                  stop=(j == 7),
                              )
                      for u, (s, c) in enumerate(units):
                          oo = po2.tile([P, 512], F32, tag="oo")
                          nc.vector.tensor_tensor(
                              oo[:], psos[u][:], bo_bc[:, ts(c, 512)], ADD
                          )
                          nc.sync.dma_start(out.ap()[ts(s, P), ts(c, 512)], oo[:])

    nc.compile()
    return nc


def make_host_inputs(q, k, v, mask, Wq, bq, Wk, bk, Wv, bv, Wo, bo):
    """Full inputs -> list of 8 per-core input dicts."""
    q = np.asarray(q, np.float32)
    k = np.asarray(k, np.float32)
    v = np.asarray(v, np.float32)
    mask = np.asarray(mask)
    Wq = np.asarray(Wq, np.float32)
    Wk = np.asarray(Wk, np.float32)
    Wv = np.asarray(Wv, np.float32)
    Wo = np.asarray(Wo, np.float32)
    bq = np.asarray(bq, np.float32)
    bk = np.asarray(bk, np.float32)
    bv = np.asarray(bv, np.float32)
    bo = np.asarray(bo, np.float32)

    def b16(a):
        return np.ascontiguousarray(a).astype(BF16_NP)

    # bv contributes exactly bv[hk] to each normalized attention output
    # (weights sum to den), so its effect folds into the output bias.
    bo_eff = bo + bv.reshape(HK) @ Wo
    shared = {
        "wq": b16(Wq.transpose(1, 0, 2).reshape(D, HK)),
        "wk": b16(Wk.transpose(1, 0, 2).reshape(D, HK)),
        "wv": b16(Wv.transpose(1, 0, 2).reshape(D, HK)),
        "wo": b16(Wo),
        "bq2": np.ascontiguousarray(bq.reshape(HK).reshape(8, P).T),
        "bk2": np.ascontiguousarray(bk.reshape(HK).reshape(8, P).T),
        "bor": np.ascontiguousarray(bo_eff.reshape(1, D)),
    }

    in_maps = []
    for core in range(8):
        b, j = divmod(core, 2)
        qs = q[b, j * SQ : (j + 1) * SQ, :]
        ms = mask[b, j * SQ : (j + 1) * SQ, :]
        # rank-1 correction: colsum of vh per head; row 64 = +SK on denom
        vsum = v[b].sum(axis=0)  # [D]
        cs = np.einsum("d,hdk->hk", vsum, Wv)  # [H, DK] (vh excludes bv)
        csum = np.empty((65, H), np.float32)
        csum[0:64, :] = cs.T
        csum[64, :] = float(SK)
        m = dict(shared)
        m["qT"] = b16(qs.T)
        m["kT"] = b16(k[b].T)
        m["vT"] = b16(v[b].T)
        m["mT"] = np.ascontiguousarray(ms.T).astype(np.float32).astype(FP8_NP)
        m["csum"] = np.ascontiguousarray(csum)
        in_maps.append(m)
    return in_maps


def assemble_output(results):
    """8 per-core out [SQ, D] -> full [4, 2048, 1024]."""
    B, S = 4, 2048
    full = np.empty((B, S, D), np.float32)
    for core, res in enumerate(results):
        b, j = divmod(core, 2)
        full[b, j * SQ : (j + 1) * SQ, :] = res["out"]
    return full


class CompiledSpmd:
    def __init__(self, nc: bass.Bass, n_cores: int):
        bass2jax.install_neuronx_cc_hook()
        assert nc.dbg_addr is None, "build with debug=False"
        partition_name = (
            nc.partition_id_tensor.name if nc.partition_id_tensor else None
        )
        in_names, out_names, out_avals, zero_outs = [], [], [], []
        for alloc in nc.m.functions[0].allocations:
            if not isinstance(alloc, mybir.MemoryLocationSet):
                continue
            name = alloc.memorylocations[0].name
            if alloc.kind == "ExternalInput":
                if name != partition_name:
                    in_names.append(name)
            elif alloc.kind == "ExternalOutput":
                shape = tuple(alloc.tensor_shape)
                dtype = mybir.dt.np(alloc.dtype)
                out_names.append(name)
                out_avals.append(jax.core.ShapedArray(shape, dtype))
                zero_outs.append(np.zeros(shape, dtype))
        n_params = len(in_names)
        n_outs = len(out_avals)
        all_in_names = list(in_names) + list(out_names)
        if partition_name is not None:
            all_in_names.append(partition_name)

        def _body(*args):
            operands = list(args)
            if partition_name is not None:
                operands.append(bass2jax.partition_id_tensor())
            outs = bass2jax._bass_exec_p.bind(
                *operands,
                out_avals=tuple(out_avals),
                in_names=tuple(all_in_names),
                out_names=tuple(out_names),
                lowering_input_output_aliases=(),
                sim_require_finite=True,
                sim_require_nnan=True,
                nc=nc,
            )
            return tuple(outs)

        devices = jax.devices()[:n_cores]
        assert len(devices) == n_cores
        mesh = Mesh(np.asarray(devices), ("core",))
        self._mesh = mesh
        donate = tuple(range(n_params, n_params + n_outs))
        self._sharded = jax.jit(
            shard_map(
                _body,
                mesh=mesh,
                in_specs=(PartitionSpec("core"),) * (n_params + n_outs),
                out_specs=(PartitionSpec("core"),) * n_outs,
                check_rep=False,
            ),
            donate_argnums=donate,
            keep_unused=True,
        )
        self.in_names = in_names
        self.out_names = out_names
        self.out_avals = out_avals
        self.zero_outs = zero_outs
        self.n_cores = n_cores

    def _concat_inputs(self, in_maps):
        per_core = [[np.asarray(m[n]) for n in self.in_names] for m in in_maps]
        return [
            np.concatenate([per_core[c][i] for c in range(self.n_cores)], axis=0)
            for i in range(len(self.in_names))
        ]

    def run(self, in_maps, repeats: int = 1):
        """Returns (results_per_core, wall_times_s list of len repeats).

        Inputs and donated zero-output buffers are device_put outside the
        timed region, so wall time ~= dispatch + NEFF execution.
        """
        from jax.sharding import NamedSharding

        mesh = self._mesh
        shard = NamedSharding(mesh, PartitionSpec("core"))
        concat_in = [
            jax.device_put(a, shard) for a in self._concat_inputs(in_maps)
        ]
        rep_zeros = [
            [
                jax.device_put(
                    np.zeros((self.n_cores * z.shape[0], *z.shape[1:]), z.dtype),
                    shard,
                )
                for z in self.zero_outs
            ]
            for _ in range(repeats)
        ]
        jax.block_until_ready(concat_in)
        jax.block_until_ready(rep_zeros)
        times = []
        out_arrs = None
        for r in range(repeats):
            t0 = time.perf_counter()
            out_arrs = self._sharded(*concat_in, *rep_zeros[r])
            jax.block_until_ready(out_arrs)
            times.append(time.perf_counter() - t0)
        results = [
            {
                name: np.asarray(out_arrs[i]).reshape(
                    self.n_cores, *self.out_avals[i].shape
                )[c]
                for i, name in enumerate(self.out_names)
            }
            for c in range(self.n_cores)
        ]
        return results, times


_COMPILED = None


def _get_compiled():
    global _COMPILED
    if _COMPILED is None:
        nc = build_mha()
        _COMPILED = CompiledSpmd(nc, 8)
    return _COMPILED


def kernel(**inputs) -> np.ndarray:
    comp = _get_compiled()
    in_maps = make_host_inputs(**inputs)
    results, _ = comp.run(in_maps, repeats=1)
    return assemble_output(results)
